# revision 28
# baseline (speedup 1.0000x reference)
"""Multi-head attention (B=2, L=2048, D=1024, H=16) on 8 trn2 NeuronCores.

Sharding: tensor-parallel over heads - 2 heads per core. Each core computes
q/k/v projections for its 2 heads, the attention for those heads, and a
row-parallel partial of the output projection (transposed). The host sums
the 8 partials (the "all-reduce") and adds the biases that were folded out
of the device kernel (bv folded through Wo, plus bo).

Device schedule: the kernel is paced by the ACT engine's exp throughput
(one [128,1024] exp per 128-column k-tile, (1024+352)/1.2GHz = 1147ns
each; 128 k-tiles total = 147us floor). Everything else hides inside that
window:

  - Attention runs as 8 single-head units of 16 k-tiles. Per k-tile the PE
    does two 512-col logits matmuls, a filler matmul, and a PV pair, all
    under ACT's ~1.15us exp.
  - PSUM (8 banks): logits double-buffer "pl" 2x[128,1024]f32 (4 banks) +
    PV accumulator "pv" (2 banks) + two [128,512] filler slots (2 banks).
  - PV lags SIX k-tiles: exp(k) completing releases both PV(k) and the
    pl-slot WAR for logits(k+2); the lag keeps released-but-queued PV work
    out of the release->logits->exp critical chain (otherwise every other
    exp eats a full PE round-trip). The last six PVs of a unit plus the
    pv-draining epilogue carry over into the next unit's first k-tiles.
  - Batch-0 projections run up front (P0); batch-1 projections, the va
    transposes, and batch-0 out-projection ride as per-k-tile filler.
  - Softmax epilogue per unit is DVE-only (pv -> sbuf copy, then
    reciprocal_approx_fast + cross-partition swap DMA + one deferred
    normalize-mul), so ACT never switches activation tables.
  - va packing: [v|ones] for head 0, [ones|v] for head 1, so the PV matmul
    also produces the softmax denominator in the free half of the
    partitions (the ones columns ride in the stationary M dim for free).
  - Tail out-projection copies are split between DVE and ACT.
"""

import numpy as np
import ml_dtypes

import concourse.bass as bass
import concourse.mybir as mybir
import concourse.tile as tile
from concourse import bacc
from concourse.bass_utils import run_bass_kernel_spmd
from concourse.masks import make_identity

B, L, D, H = 2, 2048, 1024, 16
HD = D // H              # 64 head dim
N_CORES = 8
HPC = H // N_CORES       # 2 heads per core
DK = HPC * HD            # 128 local qkv feature dim
R = B * L                # 4096 rows
KC = D // 128            # 8 contraction chunks for the projections
NB = 1024                # q-block width (one attention unit)
NRC = R // NB            # 4 row chunks
NU = L // NB             # 2 attention units per batch per head
NKT = L // 128           # 16 k tiles per batch
NRT = R // 128           # 32 row tiles
SCALE = HD ** -0.5

BF16 = mybir.dt.bfloat16
F32 = mybir.dt.float32
Act = mybir.ActivationFunctionType

_BF16_NP = ml_dtypes.bfloat16


def _body(tc, nc, xt_d, wqt_d, wkt_d, wvt_d, bq_d, bk_d, wot_d, out_d):
    with (
        tc.tile_pool(name="consts", bufs=1) as constp,
        tc.tile_pool(name="bigs", bufs=1) as bigs,
        tc.tile_pool(name="work", bufs=1) as work,
        tc.tile_pool(name="psum", bufs=1, space="PSUM") as psum,
    ):
        def mm2(ps, lhsT, rhs, start, stop):
            # one weight load, two pipelined 512-wide matmuls (psum bank limit)
            for s in (slice(0, 512), slice(512, NB)):
                nc.tensor.matmul(ps[:, s], lhsT=lhsT, rhs=rhs[:, s], start=start, stop=stop)

        # ---- load weights / biases ----
        wq_sb = constp.tile([128, KC, DK], BF16)
        wk_sb = constp.tile([128, KC, DK], BF16)
        wv_sb = constp.tile([128, KC, DK], BF16)
        wot_sb = constp.tile([DK, D], BF16)
        bq_sb = constp.tile([DK, 1], F32)
        bk_sb = constp.tile([DK, 1], F32)
        ident = constp.tile([128, 128], BF16)
        zeros = constp.tile([128, 128], BF16)
        nc.gpsimd.memset(zeros[:], 0.0)
        nc.sync.dma_start(out=wk_sb, in_=wkt_d[:])
        make_identity(nc, ident)

        # ---- load X.T ----
        # 32 column-major pieces: all 8 contraction chunks of row-chunk 0
        # first, then row-chunk 1, ... so each projection tile only waits for
        # its own columns (subtile deps) instead of the full 8MB transfer.
        # rc0 pieces go out first (split across the sync and scalar DMA
        # queues to halve the ~565ns/issue serialization) so kt-rc0 can
        # start as early as possible.
        xt_sb = [bigs.tile([128, R], BF16, name=f"xt{c}") for c in range(KC)]

        def xt_piece(c, rc, eng):
            cols = slice(rc * NB, (rc + 1) * NB)
            eng.dma_start(
                out=xt_sb[c][:, cols],
                in_=xt_d[c * 128 : (c + 1) * 128, cols],
            )

        for c in range(KC):
            xt_piece(c, 0, nc.sync if c % 2 == 0 else nc.scalar)
        nc.sync.dma_start(out=wv_sb, in_=wvt_d[:])
        nc.scalar.dma_start(out=wq_sb, in_=wqt_d[:])
        nc.sync.dma_start(out=bk_sb, in_=bk_d[:])
        nc.scalar.dma_start(out=bq_sb, in_=bq_d[:])
        for c in range(KC):
            xt_piece(c, 1, nc.sync if c % 2 == 0 else nc.scalar)
        nc.sync.dma_start(out=wot_sb, in_=wot_d[:])
        for rc in (2, 3):
            for c in range(KC):
                xt_piece(c, rc, nc.sync if c % 2 == 0 else nc.scalar)

        qt = bigs.tile([DK, R], BF16)
        kt = bigs.tile([DK, R], BF16)
        vt = bigs.tile([DK, R], BF16)
        yt = bigs.tile([DK, R], BF16)
        # va[h]: per 128-row k tile, [v_h | ones] for h0 and [ones | v_h] for
        # h1; the ones columns make the PV matmul also emit the softmax
        # denominator (h0: partitions 64:128, h1: partitions 0:64).
        va = [bigs.tile([128, R], BF16, name=f"va{h}") for h in range(HPC)]
        for h in range(HPC):
            nc.gpsimd.memset(va[h][:], 1.0)

        # ---- projection helpers ----
        def proj_drain(pp, dest, cols, bsb):
            if bsb is not None:
                nc.vector.tensor_scalar_add(out=dest[:, cols], in0=pp, scalar1=bsb)
            else:
                nc.vector.tensor_copy(out=dest[:, cols], in_=pp)

        def emit_proj_tile(wsb, bsb, dest, rc, dummies=False):
            # P0 only: full-width tile through the "pl" slots. `dummies`
            # interleaves warm-keepers so the PE p-state holds through the
            # xt-piece arrival waits.
            pp = psum.tile([128, NB], F32, tag="pl", bufs=2, name="pp")
            for c in range(KC):
                mm2(pp, wsb[:, c, :], xt_sb[c][:, rc * NB : (rc + 1) * NB],
                    start=(c == 0), stop=(c == KC - 1))
                if dummies:
                    nc.tensor.matmul(
                        warm[:, 0:512], lhsT=zeros, rhs=wk_sb[:, 0:4, :],
                        start=False, stop=False, skip_group_check=True,
                    )
            proj_drain(pp, dest, slice(rc * NB, (rc + 1) * NB), bsb)

        def emit_va_tile(t):
            # transpose one 128-row tile of vt into the va tiles
            pt = psum.tile([128, 128], BF16, tag="fil", bufs=2, name="pt")
            nc.tensor.transpose(pt, vt[:, t * 128 : (t + 1) * 128], ident)
            # h0 va block is [v | ones]; h1 va block is [ones | v]
            nc.vector.tensor_copy(
                out=va[0][:, t * 128 : t * 128 + HD], in_=pt[:, 0:HD]
            )
            nc.vector.tensor_copy(
                out=va[1][:, t * 128 + HD : (t + 1) * 128], in_=pt[:, HD:DK]
            )

        # ---- PE warmup ----
        # ~25 zero-weight matmuls streaming resident data keep the PE busy
        # from ~2us (weights arrival) until the first xt pieces land, so the
        # p-state ramp reaches 2.4GHz before the projections start.
        warm = psum.tile([128, NB], F32, tag="pl", bufs=2, name="warm")
        for di in range(25):
            nc.tensor.matmul(
                warm[:, 0:512], lhsT=zeros, rhs=wk_sb[:, 0:4, :],
                start=(di == 0), stop=False, skip_group_check=True,
            )

        # ---- P0: minimal prefix before the exp stream can start ----
        # (qt rc2 last: it waits on the rc2 xt pieces anyway)
        emit_proj_tile(wk_sb, bk_sb, kt, 0, dummies=True)
        emit_proj_tile(wv_sb, None, vt, 0, dummies=True)
        emit_proj_tile(wq_sb, bq_sb, qt, 0)
        emit_proj_tile(wk_sb, bk_sb, kt, 1, dummies=True)
        emit_proj_tile(wv_sb, None, vt, 1)
        for t in range(NKT):  # va for batch 0
            emit_va_tile(t)
        emit_proj_tile(wq_sb, bq_sb, qt, 2)  # needed by unit 4 = (b1,u0,h0)

        # ---- filler step lists per attention unit ----
        # Half-major projection filler: one [128,512] half-tile (1 psum
        # bank) accumulates its 8 chunks then drains, so only ONE of the two
        # "fil" slots is pinned at a time (the other rotates pt / po tiles).
        def proj_steps(wsb, bsb, dest, rc):
            state = {}
            steps = []
            for half in (0, 1):
                cols = slice(rc * NB + half * 512, rc * NB + (half + 1) * 512)
                for c in range(KC):
                    def step(c=c, cols=cols, last=(c == KC - 1)):
                        if c == 0:
                            state["pp"] = psum.tile(
                                [128, 512], F32, tag="fil", bufs=2, name="fp"
                            )
                        pp = state["pp"]
                        nc.tensor.matmul(
                            pp, lhsT=wsb[:, c, :], rhs=xt_sb[c][:, cols],
                            start=(c == 0), stop=(c == KC - 1),
                        )
                        if last:
                            proj_drain(pp, dest, cols, bsb)
                    steps.append(step)
            return steps

        def emit_outproj_half(ofb, half, qc, copy_eng="dve", tag="fil"):
            # half-width (1 psum bank) so the fil-slot WAR waits on a ~820ns
            # copy, which fits under the exp period when used as filler
            qh = slice(qc.start + half * 512, qc.start + (half + 1) * 512)
            po = psum.tile([128, 512], F32, tag=tag, bufs=2, name="po")
            nc.tensor.matmul(
                po, lhsT=wot_sb[:, ofb * 128 : (ofb + 1) * 128],
                rhs=yt[:, qh], start=True, stop=True,
            )
            ost = work.tile([128, 512], F32, tag="ost", bufs=6, name="ost")
            if copy_eng == "act":
                nc.scalar.copy(out=ost, in_=po)
            else:
                nc.vector.tensor_copy(out=ost, in_=po)
            nc.gpsimd.dma_start(
                out=out_d[ofb * 128 : (ofb + 1) * 128, qh], in_=ost
            )

        def outproj_steps(pair, copy_engs=("dve", "dve"), tags=("fil", "fil")):
            b, u = pair
            qc = slice(b * L + u * NB, b * L + (u + 1) * NB)
            items = []
            for i, (ofb, half) in enumerate(
                (ofb, half) for ofb in range(8) for half in (0, 1)
            ):
                items.append(
                    lambda ofb=ofb, half=half, i=i: emit_outproj_half(
                        ofb, half, qc, copy_engs[i % len(copy_engs)],
                        tags[i % len(tags)],
                    )
                )
            return items

        # filler schedule by unit index (units: (b,u,h) h-inner).
        # Constraints: qt rc1 before u2; kt-b1 rc2 before u4-k0 logits and
        # rc3 before u4-k8; vt rc2/rc3 feed the just-in-time va-b1
        # transposes in u4/u5; qt rc3 before u6. Out-projection of batch 0
        # rides in u6/u7.
        filler = {
            0: proj_steps(wq_sb, bq_sb, qt, 1),
            1: proj_steps(wk_sb, bk_sb, kt, 2),
            2: proj_steps(wk_sb, bk_sb, kt, 3),
            3: proj_steps(wv_sb, None, vt, 2),
            4: proj_steps(wv_sb, None, vt, 3),
            5: proj_steps(wq_sb, bq_sb, qt, 3),
            6: outproj_steps((0, 0)),
            7: outproj_steps((0, 1)),
        }
        # just-in-time va-b1 transposes: (unit, k-tile) -> va row tile.
        # vt rc2 drains during u3 (halves at k7/k15); vt rc3 during u4.
        va_jit = {}
        for t in range(NKT, NRT):
            if t < 28:
                va_jit[(4, t - 16)] = t      # u4 k0..11
            else:
                va_jit[(5, t - 28)] = t      # u5 k0..3 (vt rc3 half1)

        # ---- attention ----
        LAG = 6
        units = [(b, u, h) for b in (0, 1) for u in (0, 1) for h in range(HPC)]
        pending_mul = None   # deferred normalize-mul, staged by the epilogue
        carry = []           # closures from the previous unit, 1 per k-tile
        mul_state = {}

        def emit_pending_mul():
            nonlocal pending_mul
            if pending_mul is None:
                return
            h, yun, rr, qc = pending_mul
            rows = slice(0, HD) if h == 0 else slice(HD, 128)
            nc.vector.tensor_mul(
                out=yt[rows, qc], in0=yun[rows, :], in1=rr[rows, :]
            )
            pending_mul = None

        for ui, (b, u, h) in enumerate(units):
            qc = slice(b * L + u * NB, b * L + (u + 1) * NB)
            hr = slice(h * HD, (h + 1) * HD)
            steps = filler[ui]
            si = 0

            pv = psum.tile([128, NB], F32, tag="pv", bufs=1, name="pv")
            es = {}

            def emit_pv(j, b=b, h=h, pv=pv, es=es):
                tg = b * NKT + j
                mm2(pv, va[h][:, tg * 128 : (tg + 1) * 128], es.pop(j),
                    start=(j == 0), stop=(j == NKT - 1))

            for k in range(NKT):
                if (ui, k) in va_jit:
                    emit_va_tile(va_jit[(ui, k)])
                kcols = slice(b * L + k * 128, b * L + (k + 1) * 128)
                pl = psum.tile([128, NB], F32, tag="pl", bufs=2, name="pl")
                mm2(pl, kt[hr, kcols], qt[hr, qc], True, True)
                e = work.tile([128, NB], BF16, tag="exp", bufs=8, name="e")
                nc.scalar.activation(out=e, in_=pl, func=Act.Exp, scale=SCALE)
                es[k] = e
                # one carried item from the previous unit per k-tile
                if carry:
                    carry.pop(0)()
                # filler step
                if si < len(steps):
                    steps[si]()
                    si += 1
                if k >= LAG:
                    emit_pv(k - LAG)
                # Warm-keeper: small zero-weight matmul (+0 into pv), no
                # semaphore waits - keeps the PE p-state ramp at 2.4GHz
                # through the short per-k-tile waits. Only after pv has been
                # reset by its first real write.
                if ui == 0 and k == 0:
                    for di in range(12):
                        nc.tensor.matmul(
                            pv[:, 0:512], lhsT=zeros, rhs=qt[:, 0:512],
                            start=(di == 0), stop=False, skip_group_check=True,
                        )
                elif ui == 0 or k > LAG:
                    nc.tensor.matmul(
                        pv[:, 0:128], lhsT=zeros, rhs=qt[:, 0:128],
                        start=False, stop=False, skip_group_check=True,
                    )
                # the deferred mul of the previous unit (staged by the
                # carried epilogue at k7; by k11 its rr swap DMA is done)
                if k == 11:
                    emit_pending_mul()

            # ---- stage the carry: last LAG PVs + pv-draining epilogue ----
            # pv packing: h0 = [y (0:64); den (64:128)], h1 = [den; y]
            usb_ref = {}

            def make_usb_copy(pv=pv, usb_ref=usb_ref):
                def f():
                    usb = work.tile([128, NB], F32, tag="usb", bufs=2, name="usb")
                    usb_ref["t"] = usb
                    nc.vector.tensor_copy(out=usb, in_=pv)
                return f

            def make_recip_swap(h=h, qc=qc, usb_ref=usb_ref):
                def f():
                    nonlocal pending_mul
                    usb = usb_ref["t"]
                    yrows = slice(0, HD) if h == 0 else slice(HD, 128)
                    drows = slice(HD, 128) if h == 0 else slice(0, HD)
                    rsw = work.tile([128, NB], F32, tag="rsw", bufs=2, name="rsw")
                    # full-128-partition op: custom DVE ops silently drop
                    # writes when the AP has a non-zero partition base; the
                    # y-half lanes produce garbage that nothing reads
                    nc.vector.reciprocal_approx_fast(out=rsw, in_=usb)
                    rr = work.tile([128, NB], F32, tag="rr", bufs=2, name="rr")
                    nc.sync.dma_start(out=rr[yrows, :], in_=rsw[drows, :])
                    pending_mul = (h, usb, rr, qc)
                return f

            carry = [
                (lambda j=j, f=emit_pv: f(j)) for j in range(NKT - LAG, NKT)
            ] + [make_usb_copy(), make_recip_swap()]

        # ---- tail: flush the last unit, out-projection of batch 1 ----
        for item in carry:
            item()
        # warmkeeper target for the tail (the pv slot is drained by now);
        # without these the HAM throttles the PE to half clock in the tail
        dtl = psum.tile([128, NB], F32, tag="pv", bufs=1, name="dtl")
        first_dummy = [True]

        def tail_dummy():
            nc.tensor.matmul(
                dtl[:, 0:128], lhsT=zeros, rhs=wk_sb[:, 0, :],
                start=first_dummy[0], stop=False, skip_group_check=True,
            )
            first_dummy[0] = False

        # tail po tiles cycle through THREE psum slots (fil x2 + the idle
        # second pl slot) so the matmul never waits on a copy two steps back
        for step in outproj_steps((1, 0), ("dve", "act"), ("fil", "fil", "pl")):
            step()
            tail_dummy()
            tail_dummy()
            tail_dummy()
        emit_pending_mul()
        for step in outproj_steps((1, 1), ("dve", "act"), ("fil", "fil", "pl")):
            step()
            tail_dummy()
            tail_dummy()
            tail_dummy()


def build_bass():
    nc = bacc.Bacc("TRN2", target_bir_lowering=False, debug=False)
    xt_d = nc.dram_tensor("xt", [D, R], BF16, kind="ExternalInput")
    wqt_d = nc.dram_tensor("wqt", [128, KC, DK], BF16, kind="ExternalInput")
    wkt_d = nc.dram_tensor("wkt", [128, KC, DK], BF16, kind="ExternalInput")
    wvt_d = nc.dram_tensor("wvt", [128, KC, DK], BF16, kind="ExternalInput")
    bq_d = nc.dram_tensor("bq", [DK, 1], F32, kind="ExternalInput")
    bk_d = nc.dram_tensor("bk", [DK, 1], F32, kind="ExternalInput")
    wot_d = nc.dram_tensor("wot", [DK, D], BF16, kind="ExternalInput")
    out_d = nc.dram_tensor("out", [D, R], F32, kind="ExternalOutput")
    with tile.TileContext(nc) as tc:
        _body(tc, nc, xt_d, wqt_d, wkt_d, wvt_d, bq_d, bk_d, wot_d, out_d)
    nc.compile()
    return nc


_NC = None


def _get_nc():
    global _NC
    if _NC is None:
        _NC = build_bass()
    return _NC


def prepare(inputs):
    """Full inputs -> (per-core in_maps, host-side bias constant)."""
    q = np.asarray(inputs["query"], np.float32)
    Wq = np.asarray(inputs["Wq"], np.float32)
    Wk = np.asarray(inputs["Wk"], np.float32)
    Wv = np.asarray(inputs["Wv"], np.float32)
    Wo = np.asarray(inputs["Wo"], np.float32)
    bq = np.asarray(inputs["bq"], np.float32)
    bk = np.asarray(inputs["bk"], np.float32)
    bv = np.asarray(inputs["bv"], np.float32)
    bo = np.asarray(inputs["bo"], np.float32)

    X = q.reshape(R, D)
    xt = np.ascontiguousarray(X.T).astype(_BF16_NP)

    def wslice(W, hs):
        # W[hs].T laid out [p, chunk, m]: in-feat within chunk, chunk, out-feat
        return np.ascontiguousarray(
            W[hs, :].T.reshape(KC, 128, DK).transpose(1, 0, 2)
        ).astype(_BF16_NP)

    in_maps = []
    const = bo.astype(np.float64).copy()
    for c in range(N_CORES):
        hs = slice(c * DK, (c + 1) * DK)
        const += Wo[:, hs].astype(np.float64) @ bv[hs].astype(np.float64)
        in_maps.append(
            {
                "xt": xt,
                "wqt": wslice(Wq, hs),
                "wkt": wslice(Wk, hs),
                "wvt": wslice(Wv, hs),
                "bq": np.ascontiguousarray(bq[hs].reshape(DK, 1)),
                "bk": np.ascontiguousarray(bk[hs].reshape(DK, 1)),
                "wot": np.ascontiguousarray(Wo[:, hs].T).astype(_BF16_NP),
            }
        )
    return in_maps, const


def finish(results, const):
    acc = np.zeros((D, R), np.float64)
    for r in results:
        acc += np.asarray(r["out"], np.float64)
    out = acc.T + const[None, :]
    return out.astype(np.float32).reshape(B, L, D)


def run(in_maps, trace=False, **kwargs):
    nc = _get_nc()
    return run_bass_kernel_spmd(nc, in_maps, list(range(N_CORES)), trace=trace, **kwargs)


def kernel(**inputs):
    in_maps, const = prepare(inputs)
    res = run(in_maps)
    return finish(res.results, const)


# revision 30
# speedup vs baseline: 1.0201x; 1.0201x over previous
"""Multi-head attention (B=2, L=2048, D=1024, H=16) on 8 trn2 NeuronCores.

Sharding: tensor-parallel over heads - 2 heads per core. Each core computes
q/k/v projections for its 2 heads, the attention for those heads, and a
row-parallel partial of the output projection (transposed). The host sums
the 8 partials (the "all-reduce") and adds the biases that were folded out
of the device kernel (bv folded through Wo, plus bo).

Device schedule: the kernel is paced by the ACT engine's exp throughput
(one [128,1024] exp per 128-column k-tile, (1024+352)/1.2GHz = 1147ns
each; 128 k-tiles total = 147us floor). Everything else hides inside that
window:

  - Attention runs as 8 single-head units of 16 k-tiles. Per k-tile the PE
    does two 512-col logits matmuls, a filler matmul, and a PV pair, all
    under ACT's ~1.15us exp.
  - PSUM (8 banks): logits double-buffer "pl" 2x[128,1024]f32 (4 banks) +
    PV accumulator "pv" (2 banks) + two [128,512] filler slots (2 banks).
  - PV lags SIX k-tiles: exp(k) completing releases both PV(k) and the
    pl-slot WAR for logits(k+2); the lag keeps released-but-queued PV work
    out of the release->logits->exp critical chain (otherwise every other
    exp eats a full PE round-trip). The last six PVs of a unit plus the
    pv-draining epilogue carry over into the next unit's first k-tiles.
  - Batch-0 projections run up front (P0); batch-1 projections, the va
    transposes, and batch-0 out-projection ride as per-k-tile filler.
  - Softmax epilogue per unit is DVE-only (pv -> sbuf copy, then
    reciprocal_approx_fast + cross-partition swap DMA + one deferred
    normalize-mul), so ACT never switches activation tables.
  - va packing: [v|ones] for head 0, [ones|v] for head 1, so the PV matmul
    also produces the softmax denominator in the free half of the
    partitions (the ones columns ride in the stationary M dim for free).
  - Tail out-projection copies are split between DVE and ACT.
"""

import numpy as np
import ml_dtypes

import concourse.bass as bass
import concourse.mybir as mybir
import concourse.tile as tile
from concourse import bacc
from concourse.bass_utils import run_bass_kernel_spmd
from concourse.masks import make_identity

B, L, D, H = 2, 2048, 1024, 16
HD = D // H              # 64 head dim
N_CORES = 8
HPC = H // N_CORES       # 2 heads per core
DK = HPC * HD            # 128 local qkv feature dim
R = B * L                # 4096 rows
KC = D // 128            # 8 contraction chunks for the projections
NB = 1024                # q-block width (one attention unit)
NRC = R // NB            # 4 row chunks
NU = L // NB             # 2 attention units per batch per head
NKT = L // 128           # 16 k tiles per batch
NRT = R // 128           # 32 row tiles
SCALE = HD ** -0.5

BF16 = mybir.dt.bfloat16
F32 = mybir.dt.float32
Act = mybir.ActivationFunctionType

_BF16_NP = ml_dtypes.bfloat16


def _body(tc, nc, xt_d, wqt_d, wkt_d, wvt_d, bq_d, bk_d, wot_d, out_d):
    with (
        tc.tile_pool(name="consts", bufs=1) as constp,
        tc.tile_pool(name="bigs", bufs=1) as bigs,
        tc.tile_pool(name="work", bufs=1) as work,
        tc.tile_pool(name="psum", bufs=1, space="PSUM") as psum,
    ):
        def mm2(ps, lhsT, rhs, start, stop):
            # one weight load, two pipelined 512-wide matmuls (psum bank limit)
            for s in (slice(0, 512), slice(512, NB)):
                nc.tensor.matmul(ps[:, s], lhsT=lhsT, rhs=rhs[:, s], start=start, stop=stop)

        # ---- load weights / biases ----
        wq_sb = constp.tile([128, KC, DK], BF16)
        wk_sb = constp.tile([128, KC, DK], BF16)
        wv_sb = constp.tile([128, KC, DK], BF16)
        wot_sb = constp.tile([DK, D], BF16)
        bq_sb = constp.tile([DK, 1], F32)
        bk_sb = constp.tile([DK, 1], F32)
        ident = constp.tile([128, 128], BF16)
        zeros = constp.tile([128, 128], BF16)
        nc.gpsimd.memset(zeros[:], 0.0)
        nc.sync.dma_start(out=wk_sb, in_=wkt_d[:])
        make_identity(nc, ident)

        # ---- load X.T ----
        # 32 column-major pieces: all 8 contraction chunks of row-chunk 0
        # first, then row-chunk 1, ... so each projection tile only waits for
        # its own columns (subtile deps) instead of the full 8MB transfer.
        # rc0 pieces go out first (split across the sync and scalar DMA
        # queues to halve the ~565ns/issue serialization) so kt-rc0 can
        # start as early as possible.
        xt_sb = [bigs.tile([128, R], BF16, name=f"xt{c}") for c in range(KC)]

        def xt_piece(c, rc, eng):
            cols = slice(rc * NB, (rc + 1) * NB)
            eng.dma_start(
                out=xt_sb[c][:, cols],
                in_=xt_d[c * 128 : (c + 1) * 128, cols],
            )

        for c in range(KC):
            xt_piece(c, 0, nc.sync)
        nc.sync.dma_start(out=wv_sb, in_=wvt_d[:])
        nc.sync.dma_start(out=wq_sb, in_=wqt_d[:])
        nc.sync.dma_start(out=bk_sb, in_=bk_d[:])
        nc.sync.dma_start(out=bq_sb, in_=bq_d[:])
        for c in range(KC):
            xt_piece(c, 1, nc.sync)
        nc.sync.dma_start(out=wot_sb, in_=wot_d[:])
        for rc in (2, 3):
            for c in range(KC):
                xt_piece(c, rc, nc.sync)

        qt = bigs.tile([DK, R], BF16)
        kt = bigs.tile([DK, R], BF16)
        vt = bigs.tile([DK, R], BF16)
        yt = bigs.tile([DK, R], BF16)
        # va[h]: per 128-row k tile, [v_h | ones] for h0 and [ones | v_h] for
        # h1; the ones columns make the PV matmul also emit the softmax
        # denominator (h0: partitions 64:128, h1: partitions 0:64).
        va = [bigs.tile([128, R], BF16, name=f"va{h}") for h in range(HPC)]
        for h in range(HPC):
            nc.gpsimd.memset(va[h][:], 1.0)

        # ---- projection helpers ----
        def proj_drain(pp, dest, cols, bsb):
            if bsb is not None:
                nc.vector.tensor_scalar_add(out=dest[:, cols], in0=pp, scalar1=bsb)
            else:
                nc.vector.tensor_copy(out=dest[:, cols], in_=pp)

        def emit_proj_tile(wsb, bsb, dest, rc, dummies=False):
            # P0 only: full-width tile through the "pl" slots. `dummies`
            # interleaves warm-keepers so the PE p-state holds through the
            # xt-piece arrival waits.
            pp = psum.tile([128, NB], F32, tag="pl", bufs=2, name="pp")
            for c in range(KC):
                mm2(pp, wsb[:, c, :], xt_sb[c][:, rc * NB : (rc + 1) * NB],
                    start=(c == 0), stop=(c == KC - 1))
                if dummies:
                    nc.tensor.matmul(
                        warm[:, 0:512], lhsT=zeros, rhs=wk_sb[:, 0:4, :],
                        start=False, stop=False, skip_group_check=True,
                    )
            proj_drain(pp, dest, slice(rc * NB, (rc + 1) * NB), bsb)

        def emit_va_tile(t):
            # transpose one 128-row tile of vt into the va tiles
            pt = psum.tile([128, 128], BF16, tag="fil", bufs=2, name="pt")
            nc.tensor.transpose(pt, vt[:, t * 128 : (t + 1) * 128], ident)
            # h0 va block is [v | ones]; h1 va block is [ones | v]
            nc.vector.tensor_copy(
                out=va[0][:, t * 128 : t * 128 + HD], in_=pt[:, 0:HD]
            )
            nc.vector.tensor_copy(
                out=va[1][:, t * 128 + HD : (t + 1) * 128], in_=pt[:, HD:DK]
            )

        # ---- PE warmup ----
        # ~25 zero-weight matmuls streaming resident data keep the PE busy
        # from ~2us (weights arrival) until the first xt pieces land, so the
        # p-state ramp reaches 2.4GHz before the projections start.
        warm = psum.tile([128, NB], F32, tag="pl", bufs=2, name="warm")
        for di in range(25):
            nc.tensor.matmul(
                warm[:, 0:512], lhsT=zeros, rhs=wk_sb[:, 0:4, :],
                start=(di == 0), stop=False, skip_group_check=True,
            )

        # ---- P0: minimal prefix before the exp stream can start ----
        # (qt rc2 last: it waits on the rc2 xt pieces anyway)
        emit_proj_tile(wk_sb, bk_sb, kt, 0)
        emit_proj_tile(wv_sb, None, vt, 0)
        emit_proj_tile(wq_sb, bq_sb, qt, 0)
        emit_proj_tile(wk_sb, bk_sb, kt, 1)
        emit_proj_tile(wv_sb, None, vt, 1)
        for t in range(NKT):  # va for batch 0
            emit_va_tile(t)
        emit_proj_tile(wq_sb, bq_sb, qt, 2)  # needed by unit 4 = (b1,u0,h0)

        # ---- filler step lists per attention unit ----
        # Half-major projection filler: one [128,512] half-tile (1 psum
        # bank) accumulates its 8 chunks then drains, so only ONE of the two
        # "fil" slots is pinned at a time (the other rotates pt / po tiles).
        def proj_steps(wsb, bsb, dest, rc):
            state = {}
            steps = []
            for half in (0, 1):
                cols = slice(rc * NB + half * 512, rc * NB + (half + 1) * 512)
                for c in range(KC):
                    def step(c=c, cols=cols, last=(c == KC - 1)):
                        if c == 0:
                            state["pp"] = psum.tile(
                                [128, 512], F32, tag="fil", bufs=2, name="fp"
                            )
                        pp = state["pp"]
                        nc.tensor.matmul(
                            pp, lhsT=wsb[:, c, :], rhs=xt_sb[c][:, cols],
                            start=(c == 0), stop=(c == KC - 1),
                        )
                        if last:
                            proj_drain(pp, dest, cols, bsb)
                    steps.append(step)
            return steps

        def emit_outproj_half(ofb, half, qc, copy_eng="dve", tag="fil"):
            # half-width (1 psum bank) so the fil-slot WAR waits on a ~820ns
            # copy, which fits under the exp period when used as filler
            qh = slice(qc.start + half * 512, qc.start + (half + 1) * 512)
            po = psum.tile([128, 512], F32, tag=tag, bufs=2, name="po")
            nc.tensor.matmul(
                po, lhsT=wot_sb[:, ofb * 128 : (ofb + 1) * 128],
                rhs=yt[:, qh], start=True, stop=True,
            )
            ost = work.tile([128, 512], F32, tag="ost", bufs=6, name="ost")
            if copy_eng == "act":
                nc.scalar.copy(out=ost, in_=po)
            else:
                nc.vector.tensor_copy(out=ost, in_=po)
            nc.gpsimd.dma_start(
                out=out_d[ofb * 128 : (ofb + 1) * 128, qh], in_=ost
            )

        def outproj_steps(pair, copy_engs=("dve", "dve"), tags=("fil", "fil")):
            b, u = pair
            qc = slice(b * L + u * NB, b * L + (u + 1) * NB)
            items = []
            for i, (ofb, half) in enumerate(
                (ofb, half) for ofb in range(8) for half in (0, 1)
            ):
                items.append(
                    lambda ofb=ofb, half=half, i=i: emit_outproj_half(
                        ofb, half, qc, copy_engs[i % len(copy_engs)],
                        tags[i % len(tags)],
                    )
                )
            return items

        # filler schedule by unit index (units: (b,u,h) h-inner).
        # Constraints: qt rc1 before u2; kt-b1 rc2 before u4-k0 logits and
        # rc3 before u4-k8; vt rc2/rc3 feed the just-in-time va-b1
        # transposes in u4/u5; qt rc3 before u6. Out-projection of batch 0
        # rides in u6/u7.
        filler = {
            0: proj_steps(wq_sb, bq_sb, qt, 1),
            1: proj_steps(wk_sb, bk_sb, kt, 2),
            2: proj_steps(wk_sb, bk_sb, kt, 3),
            3: proj_steps(wv_sb, None, vt, 2),
            4: proj_steps(wv_sb, None, vt, 3),
            5: proj_steps(wq_sb, bq_sb, qt, 3),
            6: outproj_steps((0, 0)),
            7: outproj_steps((0, 1)),
        }
        # just-in-time va-b1 transposes: (unit, k-tile) -> va row tile.
        # vt rc2 drains during u3 (halves at k7/k15); vt rc3 during u4.
        va_jit = {}
        for t in range(NKT, NRT):
            if t < 28:
                va_jit[(4, t - 16)] = t      # u4 k0..11
            else:
                va_jit[(5, t - 28)] = t      # u5 k0..3 (vt rc3 half1)

        # ---- attention ----
        LAG = 6
        units = [(b, u, h) for b in (0, 1) for u in (0, 1) for h in range(HPC)]
        pending_mul = None   # deferred normalize-mul, staged by the epilogue
        carry = []           # closures from the previous unit, 1 per k-tile
        mul_state = {}

        def emit_pending_mul():
            nonlocal pending_mul
            if pending_mul is None:
                return
            h, yun, rr, qc = pending_mul
            rows = slice(0, HD) if h == 0 else slice(HD, 128)
            nc.vector.tensor_mul(
                out=yt[rows, qc], in0=yun[rows, :], in1=rr[rows, :]
            )
            pending_mul = None

        for ui, (b, u, h) in enumerate(units):
            qc = slice(b * L + u * NB, b * L + (u + 1) * NB)
            hr = slice(h * HD, (h + 1) * HD)
            steps = filler[ui]
            si = 0

            pv = psum.tile([128, NB], F32, tag="pv", bufs=1, name="pv")
            es = {}

            def emit_pv(j, b=b, h=h, pv=pv, es=es):
                tg = b * NKT + j
                mm2(pv, va[h][:, tg * 128 : (tg + 1) * 128], es.pop(j),
                    start=(j == 0), stop=(j == NKT - 1))

            for k in range(NKT):
                if (ui, k) in va_jit:
                    emit_va_tile(va_jit[(ui, k)])
                kcols = slice(b * L + k * 128, b * L + (k + 1) * 128)
                pl = psum.tile([128, NB], F32, tag="pl", bufs=2, name="pl")
                mm2(pl, kt[hr, kcols], qt[hr, qc], True, True)
                e = work.tile([128, NB], BF16, tag="exp", bufs=8, name="e")
                nc.scalar.activation(out=e, in_=pl, func=Act.Exp, scale=SCALE)
                es[k] = e
                # one carried item from the previous unit per k-tile
                if carry:
                    carry.pop(0)()
                # filler step
                if si < len(steps):
                    steps[si]()
                    si += 1
                if k >= LAG:
                    emit_pv(k - LAG)
                # Warm-keeper: small zero-weight matmul (+0 into pv), no
                # semaphore waits - keeps the PE p-state ramp at 2.4GHz
                # through the short per-k-tile waits. Only after pv has been
                # reset by its first real write.
                if ui == 0 and k == 0:
                    for di in range(12):
                        nc.tensor.matmul(
                            pv[:, 0:512], lhsT=zeros, rhs=qt[:, 0:512],
                            start=(di == 0), stop=False, skip_group_check=True,
                        )
                elif ui == 0 or k > LAG:
                    nc.tensor.matmul(
                        pv[:, 0:128], lhsT=zeros, rhs=qt[:, 0:128],
                        start=False, stop=False, skip_group_check=True,
                    )
                # the deferred mul of the previous unit (staged by the
                # carried epilogue at k7; by k11 its rr swap DMA is done)
                if k == 11:
                    emit_pending_mul()

            # ---- stage the carry: last LAG PVs + pv-draining epilogue ----
            # pv packing: h0 = [y (0:64); den (64:128)], h1 = [den; y]
            usb_ref = {}

            def make_usb_copy(pv=pv, usb_ref=usb_ref):
                def f():
                    usb = work.tile([128, NB], F32, tag="usb", bufs=2, name="usb")
                    usb_ref["t"] = usb
                    nc.vector.tensor_copy(out=usb, in_=pv)
                return f

            def make_recip_swap(h=h, qc=qc, usb_ref=usb_ref):
                def f():
                    nonlocal pending_mul
                    usb = usb_ref["t"]
                    yrows = slice(0, HD) if h == 0 else slice(HD, 128)
                    drows = slice(HD, 128) if h == 0 else slice(0, HD)
                    rsw = work.tile([128, NB], F32, tag="rsw", bufs=2, name="rsw")
                    # full-128-partition op: custom DVE ops silently drop
                    # writes when the AP has a non-zero partition base; the
                    # y-half lanes produce garbage that nothing reads
                    nc.vector.reciprocal_approx_fast(out=rsw, in_=usb)
                    rr = work.tile([128, NB], F32, tag="rr", bufs=2, name="rr")
                    nc.sync.dma_start(out=rr[yrows, :], in_=rsw[drows, :])
                    pending_mul = (h, usb, rr, qc)
                return f

            carry = [
                (lambda j=j, f=emit_pv: f(j)) for j in range(NKT - LAG, NKT)
            ] + [make_usb_copy(), make_recip_swap()]

        # ---- tail: flush the last unit, out-projection of batch 1 ----
        for item in carry:
            item()
        # warmkeeper target for the tail (the pv slot is drained by now);
        # without these the HAM throttles the PE to half clock in the tail
        dtl = psum.tile([128, NB], F32, tag="pv", bufs=1, name="dtl")
        first_dummy = [True]

        def tail_dummy():
            nc.tensor.matmul(
                dtl[:, 0:128], lhsT=zeros, rhs=wk_sb[:, 0, :],
                start=first_dummy[0], stop=False, skip_group_check=True,
            )
            first_dummy[0] = False

        # tail po tiles cycle through THREE psum slots (fil x2 + the idle
        # second pl slot) so the matmul never waits on a copy two steps back
        for step in outproj_steps((1, 0), ("dve", "act"), ("fil", "fil", "pl")):
            step()
            tail_dummy()
        emit_pending_mul()
        for step in outproj_steps((1, 1), ("dve", "act"), ("fil", "fil", "pl")):
            step()
            tail_dummy()


def build_bass():
    nc = bacc.Bacc("TRN2", target_bir_lowering=False, debug=False)
    xt_d = nc.dram_tensor("xt", [D, R], BF16, kind="ExternalInput")
    wqt_d = nc.dram_tensor("wqt", [128, KC, DK], BF16, kind="ExternalInput")
    wkt_d = nc.dram_tensor("wkt", [128, KC, DK], BF16, kind="ExternalInput")
    wvt_d = nc.dram_tensor("wvt", [128, KC, DK], BF16, kind="ExternalInput")
    bq_d = nc.dram_tensor("bq", [DK, 1], F32, kind="ExternalInput")
    bk_d = nc.dram_tensor("bk", [DK, 1], F32, kind="ExternalInput")
    wot_d = nc.dram_tensor("wot", [DK, D], BF16, kind="ExternalInput")
    out_d = nc.dram_tensor("out", [D, R], F32, kind="ExternalOutput")
    with tile.TileContext(nc) as tc:
        _body(tc, nc, xt_d, wqt_d, wkt_d, wvt_d, bq_d, bk_d, wot_d, out_d)
    nc.compile()
    return nc


_NC = None


def _get_nc():
    global _NC
    if _NC is None:
        _NC = build_bass()
    return _NC


def prepare(inputs):
    """Full inputs -> (per-core in_maps, host-side bias constant)."""
    q = np.asarray(inputs["query"], np.float32)
    Wq = np.asarray(inputs["Wq"], np.float32)
    Wk = np.asarray(inputs["Wk"], np.float32)
    Wv = np.asarray(inputs["Wv"], np.float32)
    Wo = np.asarray(inputs["Wo"], np.float32)
    bq = np.asarray(inputs["bq"], np.float32)
    bk = np.asarray(inputs["bk"], np.float32)
    bv = np.asarray(inputs["bv"], np.float32)
    bo = np.asarray(inputs["bo"], np.float32)

    X = q.reshape(R, D)
    xt = np.ascontiguousarray(X.T).astype(_BF16_NP)

    def wslice(W, hs):
        # W[hs].T laid out [p, chunk, m]: in-feat within chunk, chunk, out-feat
        return np.ascontiguousarray(
            W[hs, :].T.reshape(KC, 128, DK).transpose(1, 0, 2)
        ).astype(_BF16_NP)

    in_maps = []
    const = bo.astype(np.float64).copy()
    for c in range(N_CORES):
        hs = slice(c * DK, (c + 1) * DK)
        const += Wo[:, hs].astype(np.float64) @ bv[hs].astype(np.float64)
        in_maps.append(
            {
                "xt": xt,
                "wqt": wslice(Wq, hs),
                "wkt": wslice(Wk, hs),
                "wvt": wslice(Wv, hs),
                "bq": np.ascontiguousarray(bq[hs].reshape(DK, 1)),
                "bk": np.ascontiguousarray(bk[hs].reshape(DK, 1)),
                "wot": np.ascontiguousarray(Wo[:, hs].T).astype(_BF16_NP),
            }
        )
    return in_maps, const


def finish(results, const):
    acc = np.zeros((D, R), np.float64)
    for r in results:
        acc += np.asarray(r["out"], np.float64)
    out = acc.T + const[None, :]
    return out.astype(np.float32).reshape(B, L, D)


def run(in_maps, trace=False, **kwargs):
    nc = _get_nc()
    return run_bass_kernel_spmd(nc, in_maps, list(range(N_CORES)), trace=trace, **kwargs)


def kernel(**inputs):
    in_maps, const = prepare(inputs)
    res = run(in_maps)
    return finish(res.results, const)


# revision 31
# speedup vs baseline: 1.0258x; 1.0055x over previous
"""Multi-head attention (B=2, L=2048, D=1024, H=16) on 8 trn2 NeuronCores.

Sharding: tensor-parallel over heads - 2 heads per core. Each core computes
q/k/v projections for its 2 heads, the attention for those heads, and a
row-parallel partial of the output projection (transposed). The host sums
the 8 partials (the "all-reduce") and adds the biases that were folded out
of the device kernel (bv folded through Wo, plus bo).

Device schedule: the kernel is paced by the ACT engine's exp throughput
(one [128,1024] exp per 128-column k-tile, (1024+352)/1.2GHz = 1147ns
each; 128 k-tiles total = 147us floor). Everything else hides inside that
window:

  - Attention runs as 8 single-head units of 16 k-tiles. Per k-tile the PE
    does two 512-col logits matmuls, a filler matmul, and a PV pair, all
    under ACT's ~1.15us exp.
  - PSUM (8 banks): logits double-buffer "pl" 2x[128,1024]f32 (4 banks) +
    PV accumulator "pv" (2 banks) + two [128,512] filler slots (2 banks).
  - PV lags SIX k-tiles: exp(k) completing releases both PV(k) and the
    pl-slot WAR for logits(k+2); the lag keeps released-but-queued PV work
    out of the release->logits->exp critical chain (otherwise every other
    exp eats a full PE round-trip). The last six PVs of a unit plus the
    pv-draining epilogue carry over into the next unit's first k-tiles.
  - Batch-0 projections run up front (P0); batch-1 projections, the va
    transposes, and batch-0 out-projection ride as per-k-tile filler.
  - Softmax epilogue per unit is DVE-only (pv -> sbuf copy, then
    reciprocal_approx_fast + cross-partition swap DMA + one deferred
    normalize-mul), so ACT never switches activation tables.
  - va packing: [v|ones] for head 0, [ones|v] for head 1, so the PV matmul
    also produces the softmax denominator in the free half of the
    partitions (the ones columns ride in the stationary M dim for free).
  - Tail out-projection copies are split between DVE and ACT.
"""

import numpy as np
import ml_dtypes

import concourse.bass as bass
import concourse.mybir as mybir
import concourse.tile as tile
from concourse import bacc
from concourse.bass_utils import run_bass_kernel_spmd
from concourse.masks import make_identity

B, L, D, H = 2, 2048, 1024, 16
HD = D // H              # 64 head dim
N_CORES = 8
HPC = H // N_CORES       # 2 heads per core
DK = HPC * HD            # 128 local qkv feature dim
R = B * L                # 4096 rows
KC = D // 128            # 8 contraction chunks for the projections
NB = 1024                # q-block width (one attention unit)
NRC = R // NB            # 4 row chunks
NU = L // NB             # 2 attention units per batch per head
NKT = L // 128           # 16 k tiles per batch
NRT = R // 128           # 32 row tiles
SCALE = HD ** -0.5

BF16 = mybir.dt.bfloat16
F32 = mybir.dt.float32
Act = mybir.ActivationFunctionType

_BF16_NP = ml_dtypes.bfloat16


def _body(tc, nc, xt_d, wqt_d, wkt_d, wvt_d, bq_d, bk_d, wot_d, out_d):
    with (
        tc.tile_pool(name="consts", bufs=1) as constp,
        tc.tile_pool(name="bigs", bufs=1) as bigs,
        tc.tile_pool(name="work", bufs=1) as work,
        tc.tile_pool(name="psum", bufs=1, space="PSUM") as psum,
    ):
        def mm2(ps, lhsT, rhs, start, stop):
            # one weight load, two pipelined 512-wide matmuls (psum bank limit)
            for s in (slice(0, 512), slice(512, NB)):
                nc.tensor.matmul(ps[:, s], lhsT=lhsT, rhs=rhs[:, s], start=start, stop=stop)

        # ---- load weights / biases ----
        wq_sb = constp.tile([128, KC, DK], BF16)
        wk_sb = constp.tile([128, KC, DK], BF16)
        wv_sb = constp.tile([128, KC, DK], BF16)
        wot_sb = constp.tile([DK, D], BF16)
        bq_sb = constp.tile([DK, 1], F32)
        bk_sb = constp.tile([DK, 1], F32)
        ident = constp.tile([128, 128], BF16)
        zeros = constp.tile([128, 128], BF16)
        nc.gpsimd.memset(zeros[:], 0.0)
        nc.sync.dma_start(out=wk_sb, in_=wkt_d[:])
        make_identity(nc, ident)

        # ---- load X.T ----
        # 32 column-major pieces: all 8 contraction chunks of row-chunk 0
        # first, then row-chunk 1, ... so each projection tile only waits for
        # its own columns (subtile deps) instead of the full 8MB transfer.
        # rc0 pieces go out first (split across the sync and scalar DMA
        # queues to halve the ~565ns/issue serialization) so kt-rc0 can
        # start as early as possible.
        xt_sb = [bigs.tile([128, R], BF16, name=f"xt{c}") for c in range(KC)]

        def xt_piece(c, rc, eng):
            cols = slice(rc * NB, (rc + 1) * NB)
            eng.dma_start(
                out=xt_sb[c][:, cols],
                in_=xt_d[c * 128 : (c + 1) * 128, cols],
            )

        for c in range(KC):
            xt_piece(c, 0, nc.sync)
        nc.sync.dma_start(out=wv_sb, in_=wvt_d[:])
        nc.sync.dma_start(out=wq_sb, in_=wqt_d[:])
        nc.sync.dma_start(out=bk_sb, in_=bk_d[:])
        nc.sync.dma_start(out=bq_sb, in_=bq_d[:])
        for c in range(KC):
            xt_piece(c, 1, nc.sync)
        nc.sync.dma_start(out=wot_sb, in_=wot_d[:])
        for rc in (2, 3):
            for c in range(KC):
                xt_piece(c, rc, nc.sync)

        qt = bigs.tile([DK, R], BF16)
        kt = bigs.tile([DK, R], BF16)
        vt = bigs.tile([DK, R], BF16)
        yt = bigs.tile([DK, R], BF16)
        # va[h]: per 128-row k tile, [v_h | ones] for h0 and [ones | v_h] for
        # h1; the ones columns make the PV matmul also emit the softmax
        # denominator (h0: partitions 64:128, h1: partitions 0:64).
        va = [bigs.tile([128, R], BF16, name=f"va{h}") for h in range(HPC)]
        for h in range(HPC):
            nc.gpsimd.memset(va[h][:], 1.0)

        # ---- projection helpers ----
        def proj_drain(pp, dest, cols, bsb):
            if bsb is not None:
                nc.vector.tensor_scalar_add(out=dest[:, cols], in0=pp, scalar1=bsb)
            else:
                nc.vector.tensor_copy(out=dest[:, cols], in_=pp)

        def emit_proj_tile(wsb, bsb, dest, rc, dummies=False):
            # P0 only: full-width tile through the "pl" slots. `dummies`
            # interleaves warm-keepers so the PE p-state holds through the
            # xt-piece arrival waits.
            pp = psum.tile([128, NB], F32, tag="pl", bufs=2, name="pp")
            for c in range(KC):
                mm2(pp, wsb[:, c, :], xt_sb[c][:, rc * NB : (rc + 1) * NB],
                    start=(c == 0), stop=(c == KC - 1))
                if dummies:
                    nc.tensor.matmul(
                        warm[:, 0:512], lhsT=zeros, rhs=wk_sb[:, 0:4, :],
                        start=False, stop=False, skip_group_check=True,
                    )
            proj_drain(pp, dest, slice(rc * NB, (rc + 1) * NB), bsb)

        def emit_va_tile(t):
            # transpose one 128-row tile of vt into the va tiles
            pt = psum.tile([128, 128], BF16, tag="fil", bufs=2, name="pt")
            nc.tensor.transpose(pt, vt[:, t * 128 : (t + 1) * 128], ident)
            # h0 va block is [v | ones]; h1 va block is [ones | v]
            nc.vector.tensor_copy(
                out=va[0][:, t * 128 : t * 128 + HD], in_=pt[:, 0:HD]
            )
            nc.vector.tensor_copy(
                out=va[1][:, t * 128 + HD : (t + 1) * 128], in_=pt[:, HD:DK]
            )

        # ---- PE warmup ----
        # ~25 zero-weight matmuls streaming resident data keep the PE busy
        # from ~2us (weights arrival) until the first xt pieces land, so the
        # p-state ramp reaches 2.4GHz before the projections start.
        warm = psum.tile([128, NB], F32, tag="pl", bufs=2, name="warm")
        for di in range(25):
            nc.tensor.matmul(
                warm[:, 0:512], lhsT=zeros, rhs=wk_sb[:, 0:4, :],
                start=(di == 0), stop=False, skip_group_check=True,
            )

        # ---- P0: minimal prefix before the exp stream can start ----
        # (qt rc2 last: it waits on the rc2 xt pieces anyway)
        emit_proj_tile(wk_sb, bk_sb, kt, 0)
        emit_proj_tile(wv_sb, None, vt, 0)
        emit_proj_tile(wq_sb, bq_sb, qt, 0)
        emit_proj_tile(wk_sb, bk_sb, kt, 1)
        emit_proj_tile(wv_sb, None, vt, 1)
        for t in range(NKT):  # va for batch 0
            emit_va_tile(t)
        emit_proj_tile(wq_sb, bq_sb, qt, 2)  # needed by unit 4 = (b1,u0,h0)

        # ---- filler step lists per attention unit ----
        # Half-major projection filler: one [128,512] half-tile (1 psum
        # bank) accumulates its 8 chunks then drains, so only ONE of the two
        # "fil" slots is pinned at a time (the other rotates pt / po tiles).
        def proj_steps(wsb, bsb, dest, rc):
            state = {}
            steps = []
            for half in (0, 1):
                cols = slice(rc * NB + half * 512, rc * NB + (half + 1) * 512)
                for c in range(KC):
                    def step(c=c, cols=cols, last=(c == KC - 1)):
                        if c == 0:
                            state["pp"] = psum.tile(
                                [128, 512], F32, tag="fil", bufs=2, name="fp"
                            )
                        pp = state["pp"]
                        nc.tensor.matmul(
                            pp, lhsT=wsb[:, c, :], rhs=xt_sb[c][:, cols],
                            start=(c == 0), stop=(c == KC - 1),
                        )
                        if last:
                            proj_drain(pp, dest, cols, bsb)
                    steps.append(step)
            return steps

        def emit_outproj_half(ofb, half, qc, copy_eng="dve", tag="fil"):
            # half-width (1 psum bank) so the fil-slot WAR waits on a ~820ns
            # copy, which fits under the exp period when used as filler
            qh = slice(qc.start + half * 512, qc.start + (half + 1) * 512)
            po = psum.tile([128, 512], F32, tag=tag, bufs=2, name="po")
            nc.tensor.matmul(
                po, lhsT=wot_sb[:, ofb * 128 : (ofb + 1) * 128],
                rhs=yt[:, qh], start=True, stop=True,
            )
            ost = work.tile([128, 512], F32, tag="ost", bufs=6, name="ost")
            if copy_eng == "act":
                nc.scalar.copy(out=ost, in_=po)
            else:
                nc.vector.tensor_copy(out=ost, in_=po)
            nc.gpsimd.dma_start(
                out=out_d[ofb * 128 : (ofb + 1) * 128, qh], in_=ost
            )

        def outproj_steps(pair, copy_engs=("dve", "dve"), tags=("fil", "fil")):
            b, u = pair
            qc = slice(b * L + u * NB, b * L + (u + 1) * NB)
            items = []
            for i, (ofb, half) in enumerate(
                (ofb, half) for ofb in range(8) for half in (0, 1)
            ):
                items.append(
                    lambda ofb=ofb, half=half, i=i: emit_outproj_half(
                        ofb, half, qc, copy_engs[i % len(copy_engs)],
                        tags[i % len(tags)],
                    )
                )
            return items

        # filler schedule by unit index (units: (b,u,h) h-inner).
        # Constraints: qt rc1 before u2; kt-b1 rc2 before u4-k0 logits and
        # rc3 before u4-k8; vt rc2/rc3 feed the just-in-time va-b1
        # transposes in u4/u5; qt rc3 before u6. Out-projection of batch 0
        # rides in u6/u7.
        filler = {
            0: proj_steps(wq_sb, bq_sb, qt, 1),
            1: proj_steps(wk_sb, bk_sb, kt, 2),
            2: proj_steps(wk_sb, bk_sb, kt, 3),
            3: proj_steps(wv_sb, None, vt, 2),
            4: proj_steps(wv_sb, None, vt, 3),
            5: proj_steps(wq_sb, bq_sb, qt, 3),
            6: outproj_steps((0, 0)),
            7: outproj_steps((0, 1)),
        }
        # just-in-time va-b1 transposes: (unit, k-tile) -> va row tile.
        # vt rc2 drains during u3 (halves at k7/k15); vt rc3 during u4.
        va_jit = {}
        for t in range(NKT, NRT):
            if t < 28:
                va_jit[(4, t - 16)] = t      # u4 k0..11
            else:
                va_jit[(5, t - 28)] = t      # u5 k0..3 (vt rc3 half1)

        # ---- attention ----
        LAG = 6
        units = [(b, u, h) for b in (0, 1) for u in (0, 1) for h in range(HPC)]
        pending_mul = None   # deferred normalize-mul, staged by the epilogue
        carry = []           # closures from the previous unit, 1 per k-tile
        mul_state = {}

        def emit_pending_mul():
            nonlocal pending_mul
            if pending_mul is None:
                return
            h, yun, rr, qc = pending_mul
            rows = slice(0, HD) if h == 0 else slice(HD, 128)
            nc.vector.tensor_mul(
                out=yt[rows, qc], in0=yun[rows, :], in1=rr[rows, :]
            )
            pending_mul = None

        for ui, (b, u, h) in enumerate(units):
            qc = slice(b * L + u * NB, b * L + (u + 1) * NB)
            hr = slice(h * HD, (h + 1) * HD)
            steps = filler[ui]
            si = 0

            pv = psum.tile([128, NB], F32, tag="pv", bufs=1, name="pv")
            es = {}

            def emit_pv(j, b=b, h=h, pv=pv, es=es):
                tg = b * NKT + j
                mm2(pv, va[h][:, tg * 128 : (tg + 1) * 128], es.pop(j),
                    start=(j == 0), stop=(j == NKT - 1))

            for k in range(NKT):
                if (ui, k) in va_jit:
                    emit_va_tile(va_jit[(ui, k)])
                kcols = slice(b * L + k * 128, b * L + (k + 1) * 128)
                pl = psum.tile([128, NB], F32, tag="pl", bufs=2, name="pl")
                mm2(pl, kt[hr, kcols], qt[hr, qc], True, True)
                e = work.tile([128, NB], BF16, tag="exp", bufs=8, name="e")
                nc.scalar.activation(out=e, in_=pl, func=Act.Exp, scale=SCALE)
                es[k] = e
                # one carried item from the previous unit per k-tile
                if carry:
                    carry.pop(0)()
                # filler step
                if si < len(steps):
                    steps[si]()
                    si += 1
                if k >= LAG:
                    emit_pv(k - LAG)
                # Warm-up burst once at attention entry: the PE queue is
                # saturated in steady state (cadence is PE-bound), so no
                # per-k-tile warm-keepers are needed - they would stretch
                # the cadence by their own duration.
                if ui == 0 and k == 0:
                    for di in range(12):
                        nc.tensor.matmul(
                            pv[:, 0:512], lhsT=zeros, rhs=qt[:, 0:512],
                            start=(di == 0), stop=False, skip_group_check=True,
                        )
                # the deferred mul of the previous unit (staged by the
                # carried epilogue at k7; by k11 its rr swap DMA is done)
                if k == 11:
                    emit_pending_mul()

            # ---- stage the carry: last LAG PVs + pv-draining epilogue ----
            # pv packing: h0 = [y (0:64); den (64:128)], h1 = [den; y]
            usb_ref = {}

            def make_usb_copy(pv=pv, usb_ref=usb_ref):
                def f():
                    usb = work.tile([128, NB], F32, tag="usb", bufs=2, name="usb")
                    usb_ref["t"] = usb
                    nc.vector.tensor_copy(out=usb, in_=pv)
                return f

            def make_recip_swap(h=h, qc=qc, usb_ref=usb_ref):
                def f():
                    nonlocal pending_mul
                    usb = usb_ref["t"]
                    yrows = slice(0, HD) if h == 0 else slice(HD, 128)
                    drows = slice(HD, 128) if h == 0 else slice(0, HD)
                    rsw = work.tile([128, NB], F32, tag="rsw", bufs=2, name="rsw")
                    # full-128-partition op: custom DVE ops silently drop
                    # writes when the AP has a non-zero partition base; the
                    # y-half lanes produce garbage that nothing reads
                    nc.vector.reciprocal_approx_fast(out=rsw, in_=usb)
                    rr = work.tile([128, NB], F32, tag="rr", bufs=2, name="rr")
                    nc.sync.dma_start(out=rr[yrows, :], in_=rsw[drows, :])
                    pending_mul = (h, usb, rr, qc)
                return f

            carry = [
                (lambda j=j, f=emit_pv: f(j)) for j in range(NKT - LAG, NKT)
            ] + [make_usb_copy(), make_recip_swap()]

        # ---- tail: flush the last unit, out-projection of batch 1 ----
        for item in carry:
            item()
        # warmkeeper target for the tail (the pv slot is drained by now);
        # without these the HAM throttles the PE to half clock in the tail
        dtl = psum.tile([128, NB], F32, tag="pv", bufs=1, name="dtl")
        first_dummy = [True]

        def tail_dummy():
            nc.tensor.matmul(
                dtl[:, 0:128], lhsT=zeros, rhs=wk_sb[:, 0, :],
                start=first_dummy[0], stop=False, skip_group_check=True,
            )
            first_dummy[0] = False

        # tail po tiles cycle through THREE psum slots (fil x2 + the idle
        # second pl slot) so the matmul never waits on a copy two steps back
        for step in outproj_steps((1, 0), ("dve", "act"), ("fil", "fil", "pl")):
            step()
            tail_dummy()
        emit_pending_mul()
        for step in outproj_steps((1, 1), ("dve", "act"), ("fil", "fil", "pl")):
            step()
            tail_dummy()


def build_bass():
    nc = bacc.Bacc("TRN2", target_bir_lowering=False, debug=False)
    xt_d = nc.dram_tensor("xt", [D, R], BF16, kind="ExternalInput")
    wqt_d = nc.dram_tensor("wqt", [128, KC, DK], BF16, kind="ExternalInput")
    wkt_d = nc.dram_tensor("wkt", [128, KC, DK], BF16, kind="ExternalInput")
    wvt_d = nc.dram_tensor("wvt", [128, KC, DK], BF16, kind="ExternalInput")
    bq_d = nc.dram_tensor("bq", [DK, 1], F32, kind="ExternalInput")
    bk_d = nc.dram_tensor("bk", [DK, 1], F32, kind="ExternalInput")
    wot_d = nc.dram_tensor("wot", [DK, D], BF16, kind="ExternalInput")
    out_d = nc.dram_tensor("out", [D, R], F32, kind="ExternalOutput")
    with tile.TileContext(nc) as tc:
        _body(tc, nc, xt_d, wqt_d, wkt_d, wvt_d, bq_d, bk_d, wot_d, out_d)
    nc.compile()
    return nc


_NC = None


def _get_nc():
    global _NC
    if _NC is None:
        _NC = build_bass()
    return _NC


def prepare(inputs):
    """Full inputs -> (per-core in_maps, host-side bias constant)."""
    q = np.asarray(inputs["query"], np.float32)
    Wq = np.asarray(inputs["Wq"], np.float32)
    Wk = np.asarray(inputs["Wk"], np.float32)
    Wv = np.asarray(inputs["Wv"], np.float32)
    Wo = np.asarray(inputs["Wo"], np.float32)
    bq = np.asarray(inputs["bq"], np.float32)
    bk = np.asarray(inputs["bk"], np.float32)
    bv = np.asarray(inputs["bv"], np.float32)
    bo = np.asarray(inputs["bo"], np.float32)

    X = q.reshape(R, D)
    xt = np.ascontiguousarray(X.T).astype(_BF16_NP)

    def wslice(W, hs):
        # W[hs].T laid out [p, chunk, m]: in-feat within chunk, chunk, out-feat
        return np.ascontiguousarray(
            W[hs, :].T.reshape(KC, 128, DK).transpose(1, 0, 2)
        ).astype(_BF16_NP)

    in_maps = []
    const = bo.astype(np.float64).copy()
    for c in range(N_CORES):
        hs = slice(c * DK, (c + 1) * DK)
        const += Wo[:, hs].astype(np.float64) @ bv[hs].astype(np.float64)
        in_maps.append(
            {
                "xt": xt,
                "wqt": wslice(Wq, hs),
                "wkt": wslice(Wk, hs),
                "wvt": wslice(Wv, hs),
                "bq": np.ascontiguousarray(bq[hs].reshape(DK, 1)),
                "bk": np.ascontiguousarray(bk[hs].reshape(DK, 1)),
                "wot": np.ascontiguousarray(Wo[:, hs].T).astype(_BF16_NP),
            }
        )
    return in_maps, const


def finish(results, const):
    acc = np.zeros((D, R), np.float64)
    for r in results:
        acc += np.asarray(r["out"], np.float64)
    out = acc.T + const[None, :]
    return out.astype(np.float32).reshape(B, L, D)


def run(in_maps, trace=False, **kwargs):
    nc = _get_nc()
    return run_bass_kernel_spmd(nc, in_maps, list(range(N_CORES)), trace=trace, **kwargs)


def kernel(**inputs):
    in_maps, const = prepare(inputs)
    res = run(in_maps)
    return finish(res.results, const)


# revision 32
# speedup vs baseline: 1.0358x; 1.0098x over previous
"""Multi-head attention (B=2, L=2048, D=1024, H=16) on 8 trn2 NeuronCores.

Sharding: tensor-parallel over heads - 2 heads per core. Each core computes
q/k/v projections for its 2 heads, the attention for those heads, and a
row-parallel partial of the output projection (transposed). The host sums
the 8 partials (the "all-reduce") and adds the biases that were folded out
of the device kernel (bv folded through Wo, plus bo).

Device schedule: the kernel is paced by the ACT engine's exp throughput
(one [128,1024] exp per 128-column k-tile, (1024+352)/1.2GHz = 1147ns
each; 128 k-tiles total = 147us floor). Everything else hides inside that
window:

  - Attention runs as 8 single-head units of 16 k-tiles. Per k-tile the PE
    does two 512-col logits matmuls, a filler matmul, and a PV pair, all
    under ACT's ~1.15us exp.
  - PSUM (8 banks): logits double-buffer "pl" 2x[128,1024]f32 (4 banks) +
    PV accumulator "pv" (2 banks) + two [128,512] filler slots (2 banks).
  - PV lags SIX k-tiles: exp(k) completing releases both PV(k) and the
    pl-slot WAR for logits(k+2); the lag keeps released-but-queued PV work
    out of the release->logits->exp critical chain (otherwise every other
    exp eats a full PE round-trip). The last six PVs of a unit plus the
    pv-draining epilogue carry over into the next unit's first k-tiles.
  - Batch-0 projections run up front (P0); batch-1 projections, the va
    transposes, and batch-0 out-projection ride as per-k-tile filler.
  - Softmax epilogue per unit is DVE-only (pv -> sbuf copy, then
    reciprocal_approx_fast + cross-partition swap DMA + one deferred
    normalize-mul), so ACT never switches activation tables.
  - va packing: [v|ones] for head 0, [ones|v] for head 1, so the PV matmul
    also produces the softmax denominator in the free half of the
    partitions (the ones columns ride in the stationary M dim for free).
  - Tail out-projection copies are split between DVE and ACT.
"""

import numpy as np
import ml_dtypes

import concourse.bass as bass
import concourse.mybir as mybir
import concourse.tile as tile
from concourse import bacc
from concourse.bass_utils import run_bass_kernel_spmd
from concourse.masks import make_identity

B, L, D, H = 2, 2048, 1024, 16
HD = D // H              # 64 head dim
N_CORES = 8
HPC = H // N_CORES       # 2 heads per core
DK = HPC * HD            # 128 local qkv feature dim
R = B * L                # 4096 rows
KC = D // 128            # 8 contraction chunks for the projections
NB = 1024                # q-block width (one attention unit)
NRC = R // NB            # 4 row chunks
NU = L // NB             # 2 attention units per batch per head
NKT = L // 128           # 16 k tiles per batch
NRT = R // 128           # 32 row tiles
SCALE = HD ** -0.5

BF16 = mybir.dt.bfloat16
F32 = mybir.dt.float32
Act = mybir.ActivationFunctionType

_BF16_NP = ml_dtypes.bfloat16


def _body(tc, nc, xt_d, wqt_d, wkt_d, wvt_d, bq_d, bk_d, wot_d, out_d):
    with (
        tc.tile_pool(name="consts", bufs=1) as constp,
        tc.tile_pool(name="bigs", bufs=1) as bigs,
        tc.tile_pool(name="work", bufs=1) as work,
        tc.tile_pool(name="psum", bufs=1, space="PSUM") as psum,
    ):
        def mm2(ps, lhsT, rhs, start, stop):
            # one weight load, two pipelined 512-wide matmuls (psum bank limit)
            for s in (slice(0, 512), slice(512, NB)):
                nc.tensor.matmul(ps[:, s], lhsT=lhsT, rhs=rhs[:, s], start=start, stop=stop)

        # ---- load weights / biases ----
        wq_sb = constp.tile([128, KC, DK], BF16)
        wk_sb = constp.tile([128, KC, DK], BF16)
        wv_sb = constp.tile([128, KC, DK], BF16)
        wot_sb = constp.tile([DK, D], BF16)
        bq_sb = constp.tile([DK, 1], F32)
        bk_sb = constp.tile([DK, 1], F32)
        ident = constp.tile([128, 128], BF16)
        zeros = constp.tile([128, 128], BF16)
        nc.gpsimd.memset(zeros[:], 0.0)
        nc.sync.dma_start(out=wk_sb, in_=wkt_d[:])
        make_identity(nc, ident)

        # ---- load X.T ----
        # 32 column-major pieces: all 8 contraction chunks of row-chunk 0
        # first, then row-chunk 1, ... so each projection tile only waits for
        # its own columns (subtile deps) instead of the full 8MB transfer.
        # rc0 pieces go out first (split across the sync and scalar DMA
        # queues to halve the ~565ns/issue serialization) so kt-rc0 can
        # start as early as possible.
        xt_sb = [bigs.tile([128, R], BF16, name=f"xt{c}") for c in range(KC)]

        def xt_piece(c, rc, eng):
            cols = slice(rc * NB, (rc + 1) * NB)
            eng.dma_start(
                out=xt_sb[c][:, cols],
                in_=xt_d[c * 128 : (c + 1) * 128, cols],
            )

        for c in range(KC):
            xt_piece(c, 0, nc.sync)
        nc.sync.dma_start(out=wv_sb, in_=wvt_d[:])
        nc.sync.dma_start(out=wq_sb, in_=wqt_d[:])
        nc.sync.dma_start(out=bk_sb, in_=bk_d[:])
        nc.sync.dma_start(out=bq_sb, in_=bq_d[:])
        for c in range(KC):
            xt_piece(c, 1, nc.sync)
        nc.sync.dma_start(out=wot_sb, in_=wot_d[:])
        for rc in (2, 3):
            for c in range(KC):
                xt_piece(c, rc, nc.sync)

        qt = bigs.tile([DK, R], BF16)
        kt = bigs.tile([DK, R], BF16)
        vt = bigs.tile([DK, R], BF16)
        yt = bigs.tile([DK, R], BF16)
        # va[h]: per 128-row k tile, [v_h | ones] for h0 and [ones | v_h] for
        # h1; the ones columns make the PV matmul also emit the softmax
        # denominator (h0: partitions 64:128, h1: partitions 0:64).
        va = [bigs.tile([128, R], BF16, name=f"va{h}") for h in range(HPC)]
        for h in range(HPC):
            nc.gpsimd.memset(va[h][:], 1.0)

        # ---- projection helpers ----
        def proj_drain(pp, dest, cols, bsb):
            if bsb is not None:
                nc.vector.tensor_scalar_add(out=dest[:, cols], in0=pp, scalar1=bsb)
            else:
                nc.vector.tensor_copy(out=dest[:, cols], in_=pp)

        def emit_proj_tile(wsb, bsb, dest, rc, dummies=False):
            # P0 only: full-width tile through the "pl" slots. `dummies`
            # interleaves warm-keepers so the PE p-state holds through the
            # xt-piece arrival waits.
            pp = psum.tile([128, NB], F32, tag="pl", bufs=2, name="pp")
            for c in range(KC):
                mm2(pp, wsb[:, c, :], xt_sb[c][:, rc * NB : (rc + 1) * NB],
                    start=(c == 0), stop=(c == KC - 1))
                if dummies:
                    nc.tensor.matmul(
                        warm[:, 0:512], lhsT=zeros, rhs=wk_sb[:, 0:4, :],
                        start=False, stop=False, skip_group_check=True,
                    )
            proj_drain(pp, dest, slice(rc * NB, (rc + 1) * NB), bsb)

        def emit_va_tile(t):
            # transpose one 128-row tile of vt into the va tiles
            pt = psum.tile([128, 128], BF16, tag="fil", bufs=2, name="pt")
            nc.tensor.transpose(pt, vt[:, t * 128 : (t + 1) * 128], ident)
            # h0 va block is [v | ones]; h1 va block is [ones | v]
            nc.vector.tensor_copy(
                out=va[0][:, t * 128 : t * 128 + HD], in_=pt[:, 0:HD]
            )
            nc.vector.tensor_copy(
                out=va[1][:, t * 128 + HD : (t + 1) * 128], in_=pt[:, HD:DK]
            )

        # ---- PE warmup ----
        # ~25 zero-weight matmuls streaming resident data keep the PE busy
        # from ~2us (weights arrival) until the first xt pieces land, so the
        # p-state ramp reaches 2.4GHz before the projections start.
        warm = psum.tile([128, NB], F32, tag="pl", bufs=2, name="warm")
        for di in range(25):
            nc.tensor.matmul(
                warm[:, 0:512], lhsT=zeros, rhs=wk_sb[:, 0:4, :],
                start=(di == 0), stop=False, skip_group_check=True,
            )

        # ---- P0: minimal prefix before the exp stream can start ----
        # (qt rc2 last: it waits on the rc2 xt pieces anyway)
        emit_proj_tile(wk_sb, bk_sb, kt, 0)
        emit_proj_tile(wv_sb, None, vt, 0)
        emit_proj_tile(wq_sb, bq_sb, qt, 0)
        emit_proj_tile(wk_sb, bk_sb, kt, 1)
        emit_proj_tile(wv_sb, None, vt, 1)
        for t in range(NKT):  # va for batch 0
            emit_va_tile(t)
        emit_proj_tile(wq_sb, bq_sb, qt, 2)  # needed by unit 4 = (b1,u0,h0)

        # ---- filler step lists per attention unit ----
        # Half-major projection filler: one [128,512] half-tile (1 psum
        # bank) accumulates its 8 chunks then drains, so only ONE of the two
        # "fil" slots is pinned at a time (the other rotates pt / po tiles).
        def proj_steps(wsb, bsb, dest, rc):
            state = {}
            steps = []
            for half in (0, 1):
                cols = slice(rc * NB + half * 512, rc * NB + (half + 1) * 512)
                for c in range(KC):
                    def step(c=c, cols=cols, last=(c == KC - 1)):
                        if c == 0:
                            state["pp"] = psum.tile(
                                [128, 512], F32, tag="fil", bufs=2, name="fp"
                            )
                        pp = state["pp"]
                        nc.tensor.matmul(
                            pp, lhsT=wsb[:, c, :], rhs=xt_sb[c][:, cols],
                            start=(c == 0), stop=(c == KC - 1),
                        )
                        if last:
                            proj_drain(pp, dest, cols, bsb)
                    steps.append(step)
            return steps

        def emit_outproj_half(ofb, half, qc, copy_eng="dve", tag="fil"):
            # half-width (1 psum bank) so the fil-slot WAR waits on a ~820ns
            # copy, which fits under the exp period when used as filler
            qh = slice(qc.start + half * 512, qc.start + (half + 1) * 512)
            po = psum.tile([128, 512], F32, tag=tag, bufs=2, name="po")
            nc.tensor.matmul(
                po, lhsT=wot_sb[:, ofb * 128 : (ofb + 1) * 128],
                rhs=yt[:, qh], start=True, stop=True,
            )
            ost = work.tile([128, 512], F32, tag="ost", bufs=6, name="ost")
            if copy_eng == "act":
                nc.scalar.copy(out=ost, in_=po)
            else:
                nc.vector.tensor_copy(out=ost, in_=po)
            nc.gpsimd.dma_start(
                out=out_d[ofb * 128 : (ofb + 1) * 128, qh], in_=ost
            )

        def outproj_steps(pair, copy_engs=("dve", "dve"), tags=("fil", "fil")):
            b, u = pair
            qc = slice(b * L + u * NB, b * L + (u + 1) * NB)
            items = []
            for i, (ofb, half) in enumerate(
                (ofb, half) for ofb in range(8) for half in (0, 1)
            ):
                items.append(
                    lambda ofb=ofb, half=half, i=i: emit_outproj_half(
                        ofb, half, qc, copy_engs[i % len(copy_engs)],
                        tags[i % len(tags)],
                    )
                )
            return items

        # filler schedule by unit index (units: (b,u,h) h-inner).
        # Constraints: qt rc1 before u2; kt-b1 rc2 before u4-k0 logits and
        # rc3 before u4-k8; vt rc2/rc3 feed the just-in-time va-b1
        # transposes in u4/u5; qt rc3 before u6. Out-projection of batch 0
        # rides in u6/u7.
        filler = {
            0: proj_steps(wq_sb, bq_sb, qt, 1),
            1: proj_steps(wk_sb, bk_sb, kt, 2),
            2: proj_steps(wk_sb, bk_sb, kt, 3),
            3: proj_steps(wv_sb, None, vt, 2),
            4: proj_steps(wv_sb, None, vt, 3),
            5: proj_steps(wq_sb, bq_sb, qt, 3),
            6: outproj_steps((0, 0)),
            7: outproj_steps((0, 1)),
        }
        # just-in-time va-b1 transposes: (unit, k-tile) -> va row tile.
        # vt rc2 drains during u3 (halves at k7/k15); vt rc3 during u4.
        va_jit = {}
        for t in range(NKT, NRT):
            if t < 28:
                va_jit[(4, t - 16)] = t      # u4 k0..11
            else:
                va_jit[(5, t - 28)] = t      # u5 k0..3 (vt rc3 half1)

        # ---- attention ----
        LAG = 6
        units = [(b, u, h) for b in (0, 1) for u in (0, 1) for h in range(HPC)]
        pending_mul = None   # deferred normalize-mul, staged by the epilogue
        carry = []           # closures from the previous unit, 1 per k-tile
        mul_state = {}

        def emit_pending_mul():
            nonlocal pending_mul
            if pending_mul is None:
                return
            h, yun, rr, qc = pending_mul
            rows = slice(0, HD) if h == 0 else slice(HD, 128)
            nc.vector.tensor_mul(
                out=yt[rows, qc], in0=yun[rows, :], in1=rr[rows, :]
            )
            pending_mul = None

        for ui, (b, u, h) in enumerate(units):
            qc = slice(b * L + u * NB, b * L + (u + 1) * NB)
            hr = slice(h * HD, (h + 1) * HD)
            steps = filler[ui]
            si = 0

            pv = psum.tile([128, NB], F32, tag="pv", bufs=1, name="pv")
            es = {}

            def emit_pv(j, b=b, h=h, pv=pv, es=es):
                tg = b * NKT + j
                mm2(pv, va[h][:, tg * 128 : (tg + 1) * 128], es.pop(j),
                    start=(j == 0), stop=(j == NKT - 1))

            for k in range(NKT):
                if (ui, k) in va_jit:
                    emit_va_tile(va_jit[(ui, k)])
                kcols = slice(b * L + k * 128, b * L + (k + 1) * 128)
                pl = psum.tile([128, NB], F32, tag="pl", bufs=2, name="pl")
                mm2(pl, kt[hr, kcols], qt[hr, qc], True, True)
                e = work.tile([128, NB], BF16, tag="exp", bufs=8, name="e")
                nc.scalar.activation(out=e, in_=pl, func=Act.Exp, scale=SCALE)
                es[k] = e
                # one carried item from the previous unit per k-tile
                if carry:
                    carry.pop(0)()
                # filler step
                if si < len(steps):
                    steps[si]()
                    si += 1
                if k >= LAG:
                    emit_pv(k - LAG)
                # Warm-up burst once at attention entry: the PE queue is
                # saturated in steady state (cadence is PE-bound), so no
                # per-k-tile warm-keepers are needed - they would stretch
                # the cadence by their own duration.
                if ui == 0 and k == 0:
                    for di in range(12):
                        nc.tensor.matmul(
                            pv[:, 0:512], lhsT=zeros, rhs=qt[:, 0:512],
                            start=(di == 0), stop=False, skip_group_check=True,
                        )
                # the deferred mul of the previous unit (staged by the
                # carried epilogue at k7; by k11 its rr swap DMA is done)
                if k == 11:
                    emit_pending_mul()

            # ---- stage the carry: last LAG PVs + pv-draining epilogue ----
            # pv packing: h0 = [y (0:64); den (64:128)], h1 = [den; y]
            usb_ref = {}

            def make_usb_copy(pv=pv, usb_ref=usb_ref):
                def f():
                    usb = work.tile([128, NB], F32, tag="usb", bufs=2, name="usb")
                    usb_ref["t"] = usb
                    nc.vector.tensor_copy(out=usb, in_=pv)
                return f

            def make_recip_swap(h=h, qc=qc, usb_ref=usb_ref):
                def f():
                    nonlocal pending_mul
                    usb = usb_ref["t"]
                    yrows = slice(0, HD) if h == 0 else slice(HD, 128)
                    drows = slice(HD, 128) if h == 0 else slice(0, HD)
                    rsw = work.tile([128, NB], F32, tag="rsw", bufs=2, name="rsw")
                    # full-128-partition op: custom DVE ops silently drop
                    # writes when the AP has a non-zero partition base; the
                    # y-half lanes produce garbage that nothing reads
                    nc.vector.reciprocal_approx_fast(out=rsw, in_=usb)
                    rr = work.tile([128, NB], F32, tag="rr", bufs=2, name="rr")
                    nc.sync.dma_start(out=rr[yrows, :], in_=rsw[drows, :])
                    pending_mul = (h, usb, rr, qc)
                return f

            carry = [
                (lambda j=j, f=emit_pv: f(j)) for j in range(NKT - LAG, NKT)
            ] + [make_usb_copy(), make_recip_swap()]

        # ---- tail: flush the last unit, out-projection of batch 1 ----
        for item in carry:
            item()
        # warmkeeper target for the tail (the pv slot is drained by now);
        # without these the HAM throttles the PE to half clock in the tail
        dtl = psum.tile([128, NB], F32, tag="pv", bufs=1, name="dtl")
        first_dummy = [True]

        def tail_dummy():
            nc.tensor.matmul(
                dtl[:, 0:128], lhsT=zeros, rhs=wk_sb[:, 0, :],
                start=first_dummy[0], stop=False, skip_group_check=True,
            )
            first_dummy[0] = False

        # tail po tiles cycle through FOUR psum slots (fil x2 + both idle pl
        # slots) so the matmul never waits on a copy two steps back
        for step in outproj_steps((1, 0), ("dve", "act"), ("fil", "fil", "pl", "pl")):
            step()
            tail_dummy()
        emit_pending_mul()
        for step in outproj_steps((1, 1), ("dve", "act"), ("fil", "fil", "pl", "pl")):
            step()
            tail_dummy()


def build_bass():
    nc = bacc.Bacc("TRN2", target_bir_lowering=False, debug=False)
    xt_d = nc.dram_tensor("xt", [D, R], BF16, kind="ExternalInput")
    wqt_d = nc.dram_tensor("wqt", [128, KC, DK], BF16, kind="ExternalInput")
    wkt_d = nc.dram_tensor("wkt", [128, KC, DK], BF16, kind="ExternalInput")
    wvt_d = nc.dram_tensor("wvt", [128, KC, DK], BF16, kind="ExternalInput")
    bq_d = nc.dram_tensor("bq", [DK, 1], F32, kind="ExternalInput")
    bk_d = nc.dram_tensor("bk", [DK, 1], F32, kind="ExternalInput")
    wot_d = nc.dram_tensor("wot", [DK, D], BF16, kind="ExternalInput")
    out_d = nc.dram_tensor("out", [D, R], F32, kind="ExternalOutput")
    with tile.TileContext(nc) as tc:
        _body(tc, nc, xt_d, wqt_d, wkt_d, wvt_d, bq_d, bk_d, wot_d, out_d)
    nc.compile()
    return nc


_NC = None


def _get_nc():
    global _NC
    if _NC is None:
        _NC = build_bass()
    return _NC


def prepare(inputs):
    """Full inputs -> (per-core in_maps, host-side bias constant)."""
    q = np.asarray(inputs["query"], np.float32)
    Wq = np.asarray(inputs["Wq"], np.float32)
    Wk = np.asarray(inputs["Wk"], np.float32)
    Wv = np.asarray(inputs["Wv"], np.float32)
    Wo = np.asarray(inputs["Wo"], np.float32)
    bq = np.asarray(inputs["bq"], np.float32)
    bk = np.asarray(inputs["bk"], np.float32)
    bv = np.asarray(inputs["bv"], np.float32)
    bo = np.asarray(inputs["bo"], np.float32)

    X = q.reshape(R, D)
    xt = np.ascontiguousarray(X.T).astype(_BF16_NP)

    def wslice(W, hs):
        # W[hs].T laid out [p, chunk, m]: in-feat within chunk, chunk, out-feat
        return np.ascontiguousarray(
            W[hs, :].T.reshape(KC, 128, DK).transpose(1, 0, 2)
        ).astype(_BF16_NP)

    in_maps = []
    const = bo.astype(np.float64).copy()
    for c in range(N_CORES):
        hs = slice(c * DK, (c + 1) * DK)
        const += Wo[:, hs].astype(np.float64) @ bv[hs].astype(np.float64)
        in_maps.append(
            {
                "xt": xt,
                "wqt": wslice(Wq, hs),
                "wkt": wslice(Wk, hs),
                "wvt": wslice(Wv, hs),
                "bq": np.ascontiguousarray(bq[hs].reshape(DK, 1)),
                "bk": np.ascontiguousarray(bk[hs].reshape(DK, 1)),
                "wot": np.ascontiguousarray(Wo[:, hs].T).astype(_BF16_NP),
            }
        )
    return in_maps, const


def finish(results, const):
    acc = np.zeros((D, R), np.float64)
    for r in results:
        acc += np.asarray(r["out"], np.float64)
    out = acc.T + const[None, :]
    return out.astype(np.float32).reshape(B, L, D)


def run(in_maps, trace=False, **kwargs):
    nc = _get_nc()
    return run_bass_kernel_spmd(nc, in_maps, list(range(N_CORES)), trace=trace, **kwargs)


def kernel(**inputs):
    in_maps, const = prepare(inputs)
    res = run(in_maps)
    return finish(res.results, const)
